# revision 21
# baseline (speedup 1.0000x reference)
"""JPEG layer (nn_JpegLayer) Trainium2 Bass kernel, 8-core data parallel.

Host pre-shifts input by -128/255 (RGB->Y row sums to 1, chroma rows sum to
0, so this implements the JPEG level shift exactly), converts to f16, and
clamps the output on the host (+L then clip) - so the device pipeline has no
level/clamp ops at all.

Per image (per core: 4 images of [3,512,512]):
  poolH: chroma horizontal 2x pool on DVE (strided adds, f16); vertical pool
         folded into the chroma V-DCT weights -> chroma at 1/4 volume.
  P1  : V-DCT f16 matmuls, 3-accum folds the RGB->YCC color mix.
  T1  : PE transposes in f16 (1 cyc/row, half-size psum).
  P2  : H-DCT f16 (chroma already pooled).
  Q   : DVE: e = d*(1/q) f32; round via +/-1.5*2^23 trick into an f16 tile
        (integers, exact); dec = r*q in 2x-mode (all 2-byte) stored BF16.
  INV : fused IDCT-H + un-transpose in ONE bf16 matmul per 128-chunk using
        dec as lhsT (out = dec_chunk.T @ BD). Chroma rhs folds the 2x
        horizontal upsample.
  P4  : V-IDCT + YCC->RGB fold via 2-accum bf16 matmuls (chroma lhsT folds
        the vertical upsample); plain psum->bf16 evac (no clamp - host does
        it); consolidated DMA out (1 per image-channel).

f16 (11-bit mantissa) matches fp32r precision through the quantize-critical
forward path; the post-quantize inverse runs in bf16.
"""
import sys
sys.path.insert(0, '/opt/trn_rl_repo')
import numpy as np
import concourse.bacc as bacc
import concourse.bass as bass
import concourse.mybir as mybir
import concourse.tile as tile
from concourse import bass_utils

N_CORES = 8
IMG_PER_CORE = 4
H = W = 512
HT = H // 128            # 4 h-tiles per plane
LEVEL = np.float32(128.0 / 255.0)
C_ROUND = 12582912.0     # 1.5*2^23: (x+C)-C == round-half-even(x)
F32 = mybir.dt.float32
F32R = mybir.dt.float32r
F16 = mybir.dt.float16
BF16 = mybir.dt.bfloat16

RGB2YCC = np.array([[0.299, 0.587, 0.114],
                    [-0.168735892, -0.331264108, 0.5],
                    [0.5, -0.418687589, -0.081312411]], dtype=np.float32)
CB_C = np.array([0.0, -0.344136286, 1.772], dtype=np.float32)
CR_C = np.array([1.402, -0.714136286, 0.0], dtype=np.float32)


def _dct8():
    i = np.arange(8)[:, None].astype(np.float64)
    j = np.arange(8)[None, :].astype(np.float64)
    m = np.sqrt(2.0 / 8) * np.cos(np.pi * (2 * j + 1) * i / 16.0)
    m[0, :] = 1.0 / np.sqrt(8.0)
    return m.astype(np.float32)


def _blockdiag(b, reps):
    r, c = b.shape
    out = np.zeros((r * reps, c * reps), dtype=np.float32)
    for k in range(reps):
        out[k * r:(k + 1) * r, k * c:(k + 1) * c] = b
    return out


def _build_consts(quantize):
    D = _dct8()
    BD_T = _blockdiag(D.T, 16)             # [128,128] fwd 1D-DCT as lhsT
    BD = _blockdiag(D, 16)                 # [128,128] inverse
    pf8 = np.zeros((16, 8), dtype=np.float32)
    for ii in range(8):
        for dh in range(2):
            pf8[2 * ii + dh, :] = D[:, ii] * 0.5
    PF = _blockdiag(pf8, 8)                # [128, 64] V-pool+V-DCT
    pu8 = np.zeros((8, 16), dtype=np.float32)
    for jj in range(8):
        for dw in range(2):
            pu8[:, 2 * jj + dw] = D[:, jj]
    PU64 = _blockdiag(pu8, 8)              # [64, 128] IDCT + 2x upsample

    consts = {}
    for c in range(3):
        consts[f"w1y{c}"] = RGB2YCC[0, c] * BD_T
        # extra 0.5: the DVE H-pool is a sum, not a mean
        consts[f"w1c{c}"] = 0.5 * np.concatenate(
            [RGB2YCC[1, c] * PF, RGB2YCC[2, c] * PF], axis=1)  # [128,128]
    consts["bdT"] = BD_T
    consts["ident"] = np.eye(128, dtype=np.float32)
    consts["bd_bf"] = BD
    consts["pu_bf"] = _blockdiag(pu8, 16)  # [128, 256]
    consts["w4y"] = BD
    for name, cb, cr in (("R", CB_C[0], CR_C[0]), ("G", CB_C[1], CR_C[1]),
                         ("B", CB_C[2], CR_C[2])):
        m = np.zeros((128, 128), dtype=np.float32)
        m[0:64, :] = cb * PU64
        m[64:128, :] = cr * PU64
        consts[f"w4c{name}"] = m

    q = (np.round(quantize[0].astype(np.float32) * np.float32(255.0))
         / np.float32(255.0)).astype(np.float32)
    rq = (1.0 / q.astype(np.float64)).astype(np.float32)
    # freq-domain tiles are [wf partitions, (t|s)*vf cols]: value rq[u=n%8, v=p%8]
    consts["rqt"] = np.tile(rq.T, (16, 64)).astype(np.float32)   # [128,512]
    consts["qt"] = np.tile(q.T, (16, 64)).astype(np.float32)
    return consts


_CONST_SHAPES = None
_F16C = ("w1y0", "w1y1", "w1y2", "w1c0", "w1c1", "w1c2", "bdT", "ident", "qt")
_BF16C = ("bd_bf", "pu_bf", "w4y", "w4cR", "w4cG", "w4cB")


def _cdtype(name):
    if name in _F16C:
        return F16
    if name in _BF16C:
        return BF16
    return F32


def _build_nc():
    nc = bacc.Bacc("TRN2", target_bir_lowering=False, debug=False,
                   enable_asserts=False, num_devices=N_CORES)
    x_d = nc.dram_tensor("x", [IMG_PER_CORE, 3, H, W], F16,
                         kind="ExternalInput").ap()
    out_d = nc.dram_tensor("out", [IMG_PER_CORE, 3, H, W], BF16,
                           kind="ExternalOutput").ap()
    cd = {}
    for name, shape in _CONST_SHAPES.items():
        cd[name] = nc.dram_tensor(name, list(shape), _cdtype(name),
                                  kind="ExternalInput").ap()

    ACT = mybir.ActivationFunctionType
    OP = mybir.AluOpType

    with tile.TileContext(nc) as tc:
        with tc.tile_pool(name="consts", bufs=1) as cp, \
             tc.tile_pool(name="xin", bufs=4) as xb_pool, \
             tc.tile_pool(name="xpool", bufs=14) as xp_pool, \
             tc.tile_pool(name="work", bufs=6) as wp, \
             tc.tile_pool(name="decp", bufs=8) as dp, \
             tc.tile_pool(name="stage", bufs=6) as sp, \
             tc.tile_pool(name="ogp", bufs=4) as op_pool, \
             tc.tile_pool(name="ps1", bufs=1, space="PSUM") as ps1, \
             tc.tile_pool(name="ps2", bufs=2, space="PSUM") as ps2:

            cs = {}
            for name, shape in _CONST_SHAPES.items():
                cs[name] = cp.tile(list(shape), _cdtype(name),
                                   tag=f"c_{name}", name=f"c_{name}")
            early = ("w1y0", "w1y1", "w1y2", "ident", "w1c0", "w1c1", "w1c2")
            for name in early:
                nc.gpsimd.dma_start(cs[name][:], cd[name])

            for img in range(IMG_PER_CORE):
                # ---- load RGB planes ----
                # img 0: fine-grained per-t DMAs so P1 starts after 3 tiles;
                # others: one consolidated DMA per (img, color).
                X = []
                if img == 0:
                    for c in range(3):
                        xb = xb_pool.tile([128, HT, 512], F16, tag="x",
                                          name=f"x_{img}_{c}")
                        X.append(xb)
                    for t in range(HT):
                        for c in range(3):
                            src = x_d[img, c, 128 * t:128 * (t + 1), :]
                            nc.gpsimd.dma_start(X[c][:, t, :], src)
                    for name in _CONST_SHAPES:
                        if name not in early:
                            nc.gpsimd.dma_start(cs[name][:], cd[name])
                else:
                    for c in range(3):
                        xb = xb_pool.tile([128, HT, 512], F16, tag="x",
                                          name=f"x_{img}_{c}")
                        src = x_d[img, c].rearrange("(t p) w -> p t w", p=128)
                        nc.gpsimd.dma_start(xb[:, :, :], src)
                        X.append(xb)

                # ---- chroma H-pool (DVE, strided sum; /4 mean in w1c) ----
                XP = {}
                if img == 0:
                    for c in range(3):
                        for t in range(HT):
                            xpt = xp_pool.tile([128, 256], F16, tag="xp0",
                                               name=f"xp_{img}_{c}_{t}")
                            nc.vector.tensor_tensor(
                                xpt[:], X[c][:, t, 0::2], X[c][:, t, 1::2],
                                OP.add)
                            XP[c, t] = xpt
                else:
                    for c in range(3):
                        xpb = xp_pool.tile([128, HT, 256], F16, tag="xp",
                                           name=f"xp_{img}_{c}")
                        nc.vector.tensor_tensor(
                            xpb[:, :, :], X[c][:, :, 0::2], X[c][:, :, 1::2],
                            OP.add)
                        for t in range(HT):
                            XP[c, t] = None
                        XP[c, "big"] = xpb

                # ---- P1: color mix + V-DCT (f16) ----
                d1y, d1c = [], []
                for t in range(HT):
                    psY = ps2.tile([128, 512], F32, tag="p1", name="ps_p1")
                    for c in range(3):
                        nc.tensor.matmul(psY[:], cs[f"w1y{c}"][:], X[c][:, t, :],
                                         start=(c == 0), stop=(c == 2))
                    ty = wp.tile([128, 512], F16, tag="d1y",
                                 name=f"d1y_{img}_{t}")
                    nc.scalar.activation(ty[:], psY[:], ACT.Copy)
                    d1y.append(ty)
                for t in range(HT):
                    psC = ps2.tile([128, 512], F32, tag="p1", name="ps_p1")
                    for c in range(3):
                        xpc = (XP[c, t][:] if img == 0
                               else XP[c, "big"][:, t, :])
                        nc.tensor.matmul(psC[:, 0:256], cs[f"w1c{c}"][:],
                                         xpc,
                                         start=(c == 0), stop=(c == 2))
                    tcc = wp.tile([128, 256], F16, tag="d1c",
                                  name=f"d1c_{img}_{t}")
                    nc.vector.tensor_copy(tcc[:], psC[:, 0:256])
                    d1c.append(tcc)

                # ---- T1: w into partitions (f16 transposes) ----
                t1y, t1c = [], []
                for s in range(4):
                    pty = ps1.tile([128, 512], F16, tag="t1", name="ps_t1")
                    for t in range(HT):
                        nc.tensor.transpose(
                            pty[:, 128 * t:128 * (t + 1)],
                            d1y[t][:, 128 * s:128 * (s + 1)], cs["ident"][:])
                    sy = wp.tile([128, 512], F16, tag="t1y",
                                 name=f"t1y_{img}_{s}")
                    nc.scalar.activation(sy[:], pty[:], ACT.Copy)
                    t1y.append(sy)
                for s in range(2):
                    ptc = ps1.tile([128, 512], F16, tag="t1", name="ps_t1")
                    for t in range(HT):
                        nc.tensor.transpose(
                            ptc[:, 128 * t:128 * (t + 1)],
                            d1c[t][:, 128 * s:128 * (s + 1)], cs["ident"][:])
                    sc = wp.tile([128, 512], F16, tag="t1c",
                                 name=f"t1c_{img}_{s}")
                    nc.vector.tensor_copy(sc[:], ptc[:])
                    t1c.append(sc)

                # ---- P2: H-DCT + quantize ----
                decy, decc = [], []
                for k in range(6):
                    rhs = t1y[k] if k < 4 else t1c[k - 4]
                    ps = ps1.tile([128, 512], F32, tag="q", name="ps_q")
                    nc.tensor.matmul(ps[:], cs["bdT"][:], rhs[:],
                                     start=True, stop=True)
                    e = wp.tile([128, 512], F32, tag="e", name=f"e_{img}_{k}")
                    nc.vector.tensor_tensor(e[:], ps[:], cs["rqt"][:], OP.mult)
                    r = wp.tile([128, 512], F16, tag="r", name=f"r_{img}_{k}")
                    nc.vector.tensor_scalar(r[:], e[:], C_ROUND, C_ROUND,
                                            OP.add, OP.subtract)
                    dec = dp.tile([128, 512], BF16, tag="dec",
                                  name=f"dec_{img}_{k}")
                    nc.vector.tensor_tensor(dec[:], r[:], cs["qt"][:], OP.mult)
                    (decy if k < 4 else decc).append(dec)

                # ---- INV: fused IDCT-H + un-transpose (bf16, dec as lhsT) ----
                t2y, t2c = [], []
                for t in range(HT):
                    piy = ps2.tile([128, 512], F32, tag="inv", name="ps_inv")
                    for s in range(4):
                        nc.tensor.matmul(
                            piy[:, 128 * s:128 * (s + 1)],
                            decy[s][:, 128 * t:128 * (t + 1)], cs["bd_bf"][:],
                            start=True, stop=True)
                    sy = sp.tile([128, 512], BF16, tag="t2y",
                                 name=f"t2y_{img}_{t}")
                    nc.scalar.activation(sy[:], piy[:], ACT.Copy)
                    t2y.append(sy)
                    pic = ps2.tile([128, 512], F32, tag="inv", name="ps_inv")
                    for s in range(2):
                        nc.tensor.matmul(
                            pic[:, 256 * s:256 * (s + 1)],
                            decc[s][:, 128 * t:128 * (t + 1)], cs["pu_bf"][:],
                            start=True, stop=True)
                    sc = sp.tile([128, 512], BF16, tag="t2c",
                                 name=f"t2c_{img}_{t}")
                    nc.scalar.activation(sc[:], pic[:], ACT.Copy)
                    t2c.append(sc)

                # ---- P4: V-IDCT + color mix; plain evac (host clamps) ----
                for ci, cname in enumerate(("R", "G", "B")):
                    ogb = op_pool.tile([128, HT, 512], BF16, tag="og",
                                       name=f"og_{img}_{ci}")
                    for t in range(HT):
                        ps = ps2.tile([128, 512], F32, tag="o", name="ps_o")
                        nc.tensor.matmul(ps[:], cs["w4y"][:], t2y[t][:],
                                         start=True, stop=False)
                        nc.tensor.matmul(ps[:], cs[f"w4c{cname}"][:],
                                         t2c[t][:], start=False, stop=True)
                        if t < 2:
                            nc.scalar.activation(ogb[:, t, :], ps[:], ACT.Copy)
                        else:
                            nc.vector.tensor_copy(ogb[:, t, :], ps[:])
                    dst = out_d[img, ci].rearrange("(t p) w -> p t w", p=128)
                    nc.sync.dma_start(dst, ogb[:, :, :])
    nc.compile()
    return nc


_NC_CACHE = None
TRACE = False
LAST_RESULT = None


def kernel(input, quantize):
    global _NC_CACHE, _CONST_SHAPES, LAST_RESULT
    input = np.asarray(input, dtype=np.float32)
    quantize = np.asarray(quantize, dtype=np.float32)
    consts = _build_consts(quantize)
    if _CONST_SHAPES is None:
        _CONST_SHAPES = {k: v.shape for k, v in consts.items()}
    if _NC_CACHE is None:
        _NC_CACHE = _build_nc()
    nc = _NC_CACHE

    import ml_dtypes
    for name in list(consts):
        if name in _F16C:
            consts[name] = consts[name].astype(np.float16)
        elif name in _BF16C:
            consts[name] = consts[name].astype(ml_dtypes.bfloat16)

    shifted = (input - LEVEL).astype(np.float16)
    in_maps = []
    for core in range(N_CORES):
        shard = np.ascontiguousarray(
            shifted[core * IMG_PER_CORE:(core + 1) * IMG_PER_CORE])
        m = {"x": shard}
        m.update(consts)
        in_maps.append(m)
    res = bass_utils.run_bass_kernel_spmd(nc, in_maps,
                                          core_ids=list(range(N_CORES)),
                                          trace=TRACE)
    LAST_RESULT = res
    out = np.concatenate([res.results[i]["out"].astype(np.float32)
                          for i in range(N_CORES)], axis=0)
    return np.clip(out + LEVEL, 0.0, 1.0).astype(np.float32)


# revision 22
# speedup vs baseline: 1.1056x; 1.1056x over previous
"""JPEG layer (nn_JpegLayer) Trainium2 Bass kernel, 8-core data parallel.

Host pre-shifts input by -128/255 (RGB->Y row sums to 1, chroma rows sum to
0, so this implements the JPEG level shift exactly), converts to f16, and
clamps the output on the host (+L then clip) - so the device pipeline has no
level/clamp ops at all.

Per image (per core: 4 images of [3,512,512]):
  poolH: chroma horizontal 2x pool on DVE (strided adds, f16); vertical pool
         folded into the chroma V-DCT weights -> chroma at 1/4 volume.
  P1  : V-DCT f16 matmuls, 3-accum folds the RGB->YCC color mix.
  T1  : PE transposes in f16 (1 cyc/row, half-size psum).
  P2  : H-DCT f16 (chroma already pooled).
  Q   : DVE: e = d*(1/q) f32; round via +/-1.5*2^23 trick into an f16 tile
        (integers, exact); dec = r*q in 2x-mode (all 2-byte) stored BF16.
  INV : fused IDCT-H + un-transpose in ONE bf16 matmul per 128-chunk using
        dec as lhsT (out = dec_chunk.T @ BD). Chroma rhs folds the 2x
        horizontal upsample.
  P4  : V-IDCT + YCC->RGB fold via 2-accum bf16 matmuls (chroma lhsT folds
        the vertical upsample); plain psum->bf16 evac (no clamp - host does
        it); consolidated DMA out (1 per image-channel).

f16 (11-bit mantissa) matches fp32r precision through the quantize-critical
forward path; the post-quantize inverse runs in bf16.
"""
import sys
sys.path.insert(0, '/opt/trn_rl_repo')
import numpy as np
import concourse.bacc as bacc
import concourse.bass as bass
import concourse.mybir as mybir
import concourse.tile as tile
from concourse import bass_utils

N_CORES = 8
IMG_PER_CORE = 4
H = W = 512
HT = H // 128            # 4 h-tiles per plane
LEVEL = np.float32(128.0 / 255.0)
C_ROUND = 12582912.0     # 1.5*2^23: (x+C)-C == round-half-even(x)
F32 = mybir.dt.float32
F32R = mybir.dt.float32r
F16 = mybir.dt.float16
BF16 = mybir.dt.bfloat16

RGB2YCC = np.array([[0.299, 0.587, 0.114],
                    [-0.168735892, -0.331264108, 0.5],
                    [0.5, -0.418687589, -0.081312411]], dtype=np.float32)
CB_C = np.array([0.0, -0.344136286, 1.772], dtype=np.float32)
CR_C = np.array([1.402, -0.714136286, 0.0], dtype=np.float32)


def _dct8():
    i = np.arange(8)[:, None].astype(np.float64)
    j = np.arange(8)[None, :].astype(np.float64)
    m = np.sqrt(2.0 / 8) * np.cos(np.pi * (2 * j + 1) * i / 16.0)
    m[0, :] = 1.0 / np.sqrt(8.0)
    return m.astype(np.float32)


def _blockdiag(b, reps):
    r, c = b.shape
    out = np.zeros((r * reps, c * reps), dtype=np.float32)
    for k in range(reps):
        out[k * r:(k + 1) * r, k * c:(k + 1) * c] = b
    return out


def _build_consts(quantize):
    D = _dct8()
    BD_T = _blockdiag(D.T, 16)             # [128,128] fwd 1D-DCT as lhsT
    BD = _blockdiag(D, 16)                 # [128,128] inverse
    pf8 = np.zeros((16, 8), dtype=np.float32)
    for ii in range(8):
        for dh in range(2):
            pf8[2 * ii + dh, :] = D[:, ii] * 0.5
    PF = _blockdiag(pf8, 8)                # [128, 64] V-pool+V-DCT
    pu8 = np.zeros((8, 16), dtype=np.float32)
    for jj in range(8):
        for dw in range(2):
            pu8[:, 2 * jj + dw] = D[:, jj]
    PU64 = _blockdiag(pu8, 8)              # [64, 128] IDCT + 2x upsample

    consts = {}
    for c in range(3):
        consts[f"w1y{c}"] = RGB2YCC[0, c] * BD_T
        # extra 0.5: the DVE H-pool is a sum, not a mean
        consts[f"w1c{c}"] = 0.5 * np.concatenate(
            [RGB2YCC[1, c] * PF, RGB2YCC[2, c] * PF], axis=1)  # [128,128]
    consts["bdT"] = BD_T
    consts["ident"] = np.eye(128, dtype=np.float32)
    consts["bd_bf"] = BD
    consts["pu_bf"] = _blockdiag(pu8, 16)  # [128, 256]
    consts["w4y"] = BD
    for name, cb, cr in (("R", CB_C[0], CR_C[0]), ("G", CB_C[1], CR_C[1]),
                         ("B", CB_C[2], CR_C[2])):
        m = np.zeros((128, 128), dtype=np.float32)
        m[0:64, :] = cb * PU64
        m[64:128, :] = cr * PU64
        consts[f"w4c{name}"] = m

    q = (np.round(quantize[0].astype(np.float32) * np.float32(255.0))
         / np.float32(255.0)).astype(np.float32)
    rq = (1.0 / q.astype(np.float64)).astype(np.float32)
    # freq-domain tiles are [wf partitions, (t|s)*vf cols]: value rq[u=n%8, v=p%8]
    consts["rqt"] = np.tile(rq.T, (16, 64)).astype(np.float32)   # [128,512]
    consts["qt"] = np.tile(q.T, (16, 64)).astype(np.float32)
    return consts


_CONST_SHAPES = None
_F16C = ("w1y0", "w1y1", "w1y2", "w1c0", "w1c1", "w1c2", "bdT", "ident", "qt")
_BF16C = ("bd_bf", "pu_bf", "w4y", "w4cR", "w4cG", "w4cB")


def _cdtype(name):
    if name in _F16C:
        return F16
    if name in _BF16C:
        return BF16
    return F32


def _build_nc():
    nc = bacc.Bacc("TRN2", target_bir_lowering=False, debug=False,
                   enable_asserts=False, num_devices=N_CORES)
    x_d = nc.dram_tensor("x", [IMG_PER_CORE, 3, H, W], F16,
                         kind="ExternalInput").ap()
    out_d = nc.dram_tensor("out", [IMG_PER_CORE, 3, H, W], BF16,
                           kind="ExternalOutput").ap()
    cd = {}
    for name, shape in _CONST_SHAPES.items():
        cd[name] = nc.dram_tensor(name, list(shape), _cdtype(name),
                                  kind="ExternalInput").ap()

    ACT = mybir.ActivationFunctionType
    OP = mybir.AluOpType

    with tile.TileContext(nc) as tc:
        with tc.tile_pool(name="consts", bufs=1) as cp, \
             tc.tile_pool(name="xin", bufs=4) as xb_pool, \
             tc.tile_pool(name="xpool", bufs=14) as xp_pool, \
             tc.tile_pool(name="work", bufs=6) as wp, \
             tc.tile_pool(name="decp", bufs=8) as dp, \
             tc.tile_pool(name="stage", bufs=6) as sp, \
             tc.tile_pool(name="ogp", bufs=4) as op_pool, \
             tc.tile_pool(name="ps1", bufs=1, space="PSUM") as ps1, \
             tc.tile_pool(name="ps2", bufs=2, space="PSUM") as ps2:

            cs = {}
            for name, shape in _CONST_SHAPES.items():
                cs[name] = cp.tile(list(shape), _cdtype(name),
                                   tag=f"c_{name}", name=f"c_{name}")
            early = ("w1y0", "w1y1", "w1y2", "ident", "w1c0", "w1c1", "w1c2")
            for name in early:
                nc.sync.dma_start(cs[name][:], cd[name])

            for img in range(IMG_PER_CORE):
                # ---- load RGB planes ----
                # img 0: fine-grained per-t DMAs so P1 starts after 3 tiles;
                # others: one consolidated DMA per (img, color).
                X = []
                if img == 0:
                    for c in range(3):
                        xb = xb_pool.tile([128, HT, 512], F16, tag="x",
                                          name=f"x_{img}_{c}")
                        X.append(xb)
                    for t in range(HT):
                        for c in range(3):
                            src = x_d[img, c, 128 * t:128 * (t + 1), :]
                            nc.sync.dma_start(X[c][:, t, :], src)
                    for name in _CONST_SHAPES:
                        if name not in early:
                            nc.sync.dma_start(cs[name][:], cd[name])
                else:
                    for c in range(3):
                        xb = xb_pool.tile([128, HT, 512], F16, tag="x",
                                          name=f"x_{img}_{c}")
                        src = x_d[img, c].rearrange("(t p) w -> p t w", p=128)
                        nc.sync.dma_start(xb[:, :, :], src)
                        X.append(xb)

                # ---- chroma H-pool (DVE, strided sum; /4 mean in w1c) ----
                XP = {}
                if img == 0:
                    for c in range(3):
                        for t in range(HT):
                            xpt = xp_pool.tile([128, 256], F16, tag="xp0",
                                               name=f"xp_{img}_{c}_{t}")
                            nc.vector.tensor_tensor(
                                xpt[:], X[c][:, t, 0::2], X[c][:, t, 1::2],
                                OP.add)
                            XP[c, t] = xpt
                else:
                    for c in range(3):
                        xpb = xp_pool.tile([128, HT, 256], F16, tag="xp",
                                           name=f"xp_{img}_{c}")
                        nc.vector.tensor_tensor(
                            xpb[:, :, :], X[c][:, :, 0::2], X[c][:, :, 1::2],
                            OP.add)
                        for t in range(HT):
                            XP[c, t] = None
                        XP[c, "big"] = xpb

                # ---- P1: color mix + V-DCT (f16) ----
                d1y, d1c = [], []
                for t in range(HT):
                    psY = ps2.tile([128, 512], F32, tag="p1", name="ps_p1")
                    for c in range(3):
                        nc.tensor.matmul(psY[:], cs[f"w1y{c}"][:], X[c][:, t, :],
                                         start=(c == 0), stop=(c == 2))
                    ty = wp.tile([128, 512], F16, tag="d1y",
                                 name=f"d1y_{img}_{t}")
                    nc.scalar.activation(ty[:], psY[:], ACT.Copy)
                    d1y.append(ty)
                for t in range(HT):
                    psC = ps2.tile([128, 512], F32, tag="p1", name="ps_p1")
                    for c in range(3):
                        xpc = (XP[c, t][:] if img == 0
                               else XP[c, "big"][:, t, :])
                        nc.tensor.matmul(psC[:, 0:256], cs[f"w1c{c}"][:],
                                         xpc,
                                         start=(c == 0), stop=(c == 2))
                    tcc = wp.tile([128, 256], F16, tag="d1c",
                                  name=f"d1c_{img}_{t}")
                    nc.vector.tensor_copy(tcc[:], psC[:, 0:256])
                    d1c.append(tcc)

                # ---- T1: w into partitions (f16 transposes) ----
                t1y, t1c = [], []
                for s in range(4):
                    pty = ps1.tile([128, 512], F16, tag="t1", name="ps_t1")
                    for t in range(HT):
                        nc.tensor.transpose(
                            pty[:, 128 * t:128 * (t + 1)],
                            d1y[t][:, 128 * s:128 * (s + 1)], cs["ident"][:])
                    sy = wp.tile([128, 512], F16, tag="t1y",
                                 name=f"t1y_{img}_{s}")
                    nc.scalar.activation(sy[:], pty[:], ACT.Copy)
                    t1y.append(sy)
                for s in range(2):
                    ptc = ps1.tile([128, 512], F16, tag="t1", name="ps_t1")
                    for t in range(HT):
                        nc.tensor.transpose(
                            ptc[:, 128 * t:128 * (t + 1)],
                            d1c[t][:, 128 * s:128 * (s + 1)], cs["ident"][:])
                    sc = wp.tile([128, 512], F16, tag="t1c",
                                 name=f"t1c_{img}_{s}")
                    nc.scalar.activation(sc[:], ptc[:], ACT.Copy)
                    t1c.append(sc)

                # ---- P2: H-DCT + quantize ----
                decy, decc = [], []
                for k in range(6):
                    rhs = t1y[k] if k < 4 else t1c[k - 4]
                    ps = ps1.tile([128, 512], F32, tag="q", name="ps_q")
                    nc.tensor.matmul(ps[:], cs["bdT"][:], rhs[:],
                                     start=True, stop=True)
                    e = wp.tile([128, 512], F32, tag="e", name=f"e_{img}_{k}")
                    nc.vector.tensor_tensor(e[:], ps[:], cs["rqt"][:], OP.mult)
                    r = wp.tile([128, 512], F16, tag="r", name=f"r_{img}_{k}")
                    nc.vector.tensor_scalar(r[:], e[:], C_ROUND, C_ROUND,
                                            OP.add, OP.subtract)
                    dec = dp.tile([128, 512], BF16, tag="dec",
                                  name=f"dec_{img}_{k}")
                    nc.vector.tensor_tensor(dec[:], r[:], cs["qt"][:], OP.mult)
                    (decy if k < 4 else decc).append(dec)

                # ---- INV: fused IDCT-H + un-transpose (bf16, dec as lhsT) ----
                t2y, t2c = [], []
                for t in range(HT):
                    piy = ps2.tile([128, 512], F32, tag="inv", name="ps_inv")
                    for s in range(4):
                        nc.tensor.matmul(
                            piy[:, 128 * s:128 * (s + 1)],
                            decy[s][:, 128 * t:128 * (t + 1)], cs["bd_bf"][:],
                            start=True, stop=True)
                    sy = sp.tile([128, 512], BF16, tag="t2y",
                                 name=f"t2y_{img}_{t}")
                    nc.scalar.activation(sy[:], piy[:], ACT.Copy)
                    t2y.append(sy)
                    pic = ps2.tile([128, 512], F32, tag="inv", name="ps_inv")
                    for s in range(2):
                        nc.tensor.matmul(
                            pic[:, 256 * s:256 * (s + 1)],
                            decc[s][:, 128 * t:128 * (t + 1)], cs["pu_bf"][:],
                            start=True, stop=True)
                    sc = sp.tile([128, 512], BF16, tag="t2c",
                                 name=f"t2c_{img}_{t}")
                    nc.scalar.activation(sc[:], pic[:], ACT.Copy)
                    t2c.append(sc)

                # ---- P4: V-IDCT + color mix; plain evac (host clamps) ----
                for ci, cname in enumerate(("R", "G", "B")):
                    ogb = op_pool.tile([128, HT, 512], BF16, tag="og",
                                       name=f"og_{img}_{ci}")
                    for t in range(HT):
                        ps = ps2.tile([128, 512], F32, tag="o", name="ps_o")
                        nc.tensor.matmul(ps[:], cs["w4y"][:], t2y[t][:],
                                         start=True, stop=False)
                        nc.tensor.matmul(ps[:], cs[f"w4c{cname}"][:],
                                         t2c[t][:], start=False, stop=True)
                        if t < 2:
                            nc.scalar.activation(ogb[:, t, :], ps[:], ACT.Copy)
                        else:
                            nc.vector.tensor_copy(ogb[:, t, :], ps[:])
                    dst = out_d[img, ci].rearrange("(t p) w -> p t w", p=128)
                    nc.sync.dma_start(dst, ogb[:, :, :])
    nc.compile()
    return nc


_NC_CACHE = None
TRACE = False
LAST_RESULT = None


def kernel(input, quantize):
    global _NC_CACHE, _CONST_SHAPES, LAST_RESULT
    input = np.asarray(input, dtype=np.float32)
    quantize = np.asarray(quantize, dtype=np.float32)
    consts = _build_consts(quantize)
    if _CONST_SHAPES is None:
        _CONST_SHAPES = {k: v.shape for k, v in consts.items()}
    if _NC_CACHE is None:
        _NC_CACHE = _build_nc()
    nc = _NC_CACHE

    import ml_dtypes
    for name in list(consts):
        if name in _F16C:
            consts[name] = consts[name].astype(np.float16)
        elif name in _BF16C:
            consts[name] = consts[name].astype(ml_dtypes.bfloat16)

    shifted = (input - LEVEL).astype(np.float16)
    in_maps = []
    for core in range(N_CORES):
        shard = np.ascontiguousarray(
            shifted[core * IMG_PER_CORE:(core + 1) * IMG_PER_CORE])
        m = {"x": shard}
        m.update(consts)
        in_maps.append(m)
    res = bass_utils.run_bass_kernel_spmd(nc, in_maps,
                                          core_ids=list(range(N_CORES)),
                                          trace=TRACE)
    LAST_RESULT = res
    out = np.concatenate([res.results[i]["out"].astype(np.float32)
                          for i in range(N_CORES)], axis=0)
    return np.clip(out + LEVEL, 0.0, 1.0).astype(np.float32)
